# revision 2
# baseline (speedup 1.0000x reference)
"""MoE top-k routing + grouped down-proj GEMM + reduce-scatter for trn2 (8 cores).

Problem: intermediate_states [4, 2048, 1024] f16 (rank-sharded expanded-token
activations), w [4, 8, 1024, 2048] f16 (rank-sharded per-expert down-proj),
router_logits [1024, 8] f32, topk=2.  Output [4, 256, 2048] f16.

Strategy (W-stationary streaming): per expanded token tk routed to expert
e(tk): y[tk] = gate(tk) * (x_full[tk] @ W_full[e(tk)]) with x_full [TK, 4096]
(rank dim folded into contraction) and W_full[e] [4096, 2048].  Gates are
pre-applied to x on the host (f32 multiply, cast to f16), so the device does
pure GEMM.

Work unit = (expert, K-half, token-chunk<=512).  For 8 experts with c_e<=512
that is 16 units.  Units are sorted by token count and packed into U=2 "slots"
of 8 (one unit per core per slot); each slot has a common padded capacity C_s
= max count in slot, so all 8 cores run an IDENTICAL program (SPMD) perfectly
balanced by construction.  Per unit the PE runs, for each of 16 h-chunks of
128 output columns, a 16-step accumulation over K-subtiles:

    matmul(psum[128h, C], lhsT=W[128k, 128h], rhs=x[128k, C])

i.e. W is the stationary operand (a fresh [128,128] tile per matmul, loaded
via LDWEIGHTS which pipelines ahead of the running matmul through the PE's
reorder window + FWL) and the moving operand is the unit's token block, so
the matmul cost is proportional to the REAL token count (padded only to the
slot capacity), not to a fixed 128-token tile grid.  W is single-use and
streamed through a 6-buffer SBUF pool in exact consumption order (h-major),
~60 GB per-core traffic budget: W 16.8MB + x ~2.4MB + out ~2.2MB.

Evictions (PSUM f32 -> SBUF f16 copy, no scale needed) alternate between the
scalar and vector engines, one per h-chunk, grouped 4 h-chunks per output DMA
(issued on the gpsimd queue to keep the sync queue free for the W stream).
The tail after the last matmul is one eviction + one small DMA.

Host combine: partial outputs are [h, token]-major f16; host transposes,
accumulates the 2 K-half partials and the topk expert partials per token in
f32, and emits [4, 256, 2048] f16.
"""

import numpy as np

R, T_TOK, E = 4, 1024, 8
I_PR, H = 1024, 2048
K = R * I_PR            # 4096 contraction
P = 128
N_CORES = 8
NKS = 16                # K-subtiles per K-half unit (2048/128)
NHC = H // P            # 16 h-chunks
CHUNK_CAP = 512         # max tokens per unit (one PSUM bank of fp32)
OG = 4                  # h-chunks per output DMA group
NWARM = 8               # p-state warmup matmuls on a zeroed tile
W_BUFS = 6              # W stream SBUF pool depth

_prog_cache: dict[tuple, object] = {}


def _new_bacc():
    from concourse import bacc

    return bacc.Bacc(
        "TRN2",
        target_bir_lowering=False,
        debug=False,
        num_devices=N_CORES,
    )


def _build_program(caps: tuple):
    import concourse.mybir as mybir
    import concourse.tile as tile

    f16 = mybir.dt.float16
    f32 = mybir.dt.float32

    nc = _new_bacc()
    xs, ws, os_ = [], [], []
    for s, C in enumerate(caps):
        xs.append(nc.declare_dram_parameter(f"x{s}", [P, NKS * C], f16,
                                            isOutput=False))
        ws.append(nc.declare_dram_parameter(f"w{s}", [NHC, P, H], f16,
                                            isOutput=False))
        os_.append(nc.declare_dram_parameter(f"o{s}", [NHC // OG, P, OG * C],
                                             f16, isOutput=True))

    with tile.TileContext(nc) as tc:
        with tc.tile_pool(name="sb", bufs=1) as sb, \
             tc.tile_pool(name="ps", bufs=2, space="PSUM") as psp:
            # x tiles are fully resident; their DMAs ride the scalar queue
            # (4 chunks per unit so the first matmuls aren't gated on the
            # whole 1.2MB transfer), leaving the sync queue exclusively for
            # the W stream in exact consumption order.
            xt = [sb.tile([P, NKS * C], f16, name=f"x{s}", tag=f"x{s}", bufs=1)
                  for s, C in enumerate(caps)]
            for s, C in enumerate(caps):
                XQ = NKS * C // 4
                for q in range(4):
                    nc.scalar.dma_start(xt[s][:, q * XQ:(q + 1) * XQ],
                                        xs[s][:, q * XQ:(q + 1) * XQ])

            # p-state warmup: PE ramps to full clock (~3us of busy) while the
            # first W/x chunks stream in; garbage-free (zeroed operands).
            warm = sb.tile([P, 512], f16, name="warm", tag="warm", bufs=1)
            nc.vector.memset(warm[:], 0.0)
            ps_warm = psp.tile([P, 512], f32, name="psw", tag="ps", bufs=8)
            for i in range(NWARM):
                nc.tensor.matmul(ps_warm[:, :512], lhsT=warm[:, :P],
                                 rhs=warm[:], start=(i == 0),
                                 stop=(i == NWARM - 1))

            for s, C in enumerate(caps):
                ot = None
                for h in range(NHC):
                    wt = sb.tile([P, H], f16, name=f"w{s}_{h}", tag="w",
                                 bufs=W_BUFS)
                    if s == 0 and h == 0:
                        # first W slab in halves so matmuls start sooner
                        nc.sync.dma_start(wt[:, :H // 2], ws[s][h, :, :H // 2])
                        nc.sync.dma_start(wt[:, H // 2:], ws[s][h, :, H // 2:])
                    else:
                        nc.sync.dma_start(wt[:], ws[s][h, :, :])
                    ps = psp.tile([P, 512], f32, name=f"ps{s}_{h}", tag="ps",
                                  bufs=8)
                    for ks in range(NKS):
                        nc.tensor.matmul(
                            ps[:, :C],
                            lhsT=wt[:, ks * P:(ks + 1) * P],
                            rhs=xt[s][:, ks * C:(ks + 1) * C],
                            start=(ks == 0),
                            stop=(ks == NKS - 1),
                        )
                    g, hh = divmod(h, OG)
                    if hh == 0:
                        ot = sb.tile([P, OG * C], f16, name=f"o{s}_{g}",
                                     tag="o", bufs=3)
                    dst = ot[:, hh * C:(hh + 1) * C]
                    if h % 2 == 0:
                        nc.scalar.copy(dst, ps[:, :C])
                    else:
                        nc.vector.tensor_scalar_mul(dst, ps[:, :C], 1.0)
                    if hh == OG - 1:
                        nc.gpsimd.dma_start(os_[s][g, :, :], ot[:])
    nc.finalize()
    return nc


def _get_program(caps: tuple):
    if caps not in _prog_cache:
        _prog_cache[caps] = _build_program(caps)
    return _prog_cache[caps]


def _route(logits, topk):
    """numpy replica of jax.lax.top_k + softmax over selected logits."""
    idx = np.argsort(-logits, axis=-1, kind="stable")[:, :topk]      # [T, topk]
    vals = np.take_along_axis(logits, idx, axis=-1)
    mx = vals.max(-1, keepdims=True)
    gate = np.exp(vals - mx)
    gate = gate / gate.sum(-1, keepdims=True)                        # f32
    return idx, gate


def prepare(inputs):
    """Host routing + per-core input construction.

    Returns (nc, launches, combine): launches is a list of per-launch in_maps
    (one dict per core); combine(list_of_per_launch_results) -> final output.
    """
    x = np.asarray(inputs["intermediate_states"])          # [R, TK, I_PR] f16
    w = np.asarray(inputs["w"])                            # [R, E, I_PR, H] f16
    logits = np.asarray(inputs["router_logits"]).astype(np.float32)  # [T, E]
    topk = int(np.asarray(inputs["topk"]))

    T, E_ = logits.shape
    TK = T * topk
    assert x.shape == (R, TK, I_PR) and w.shape == (R, E_, I_PR, H) and E_ == E

    idx, gate = _route(logits, topk)
    flat_e = idx.reshape(-1)                               # expert of tk
    g_flat = gate.reshape(TK)
    counts = np.bincount(flat_e, minlength=E)
    starts = np.zeros(E + 1, np.int64)
    starts[1:] = np.cumsum(counts)
    order = np.argsort(flat_e, kind="stable")              # tks sorted by expert

    # pre-gated activations: y contribution of token row is gate * x row
    xf = np.ascontiguousarray(x.transpose(1, 0, 2)).reshape(TK, K)
    xg = (xf.astype(np.float32) * g_flat[:, None]).astype(np.float16)

    # build units: (expert, kh, token array)
    units = []
    for e in range(E):
        toks_e = order[starts[e]:starts[e + 1]]
        for lo in range(0, max(len(toks_e), 1), CHUNK_CAP):
            chunk = toks_e[lo:lo + CHUNK_CAP]
            for kh in range(2):
                units.append((e, kh, chunk))
    while len(units) % N_CORES:
        units.append((0, 0, np.empty(0, np.int64)))
    units.sort(key=lambda u: -len(u[2]))
    U = len(units) // N_CORES

    caps = []
    for s in range(U):
        cmax = max(len(u[2]) for u in units[s * N_CORES:(s + 1) * N_CORES])
        caps.append(max(8, -(-cmax // 8) * 8))
    caps = tuple(caps)
    nc = _get_program(caps)

    KH = K // 2
    in_maps = [dict() for _ in range(N_CORES)]
    unit_of = {}                                           # (core, slot) -> unit
    for s, C in enumerate(caps):
        for c_core in range(N_CORES):
            e, kh, toks = units[s * N_CORES + c_core]
            n = len(toks)
            unit_of[(c_core, s)] = (toks, C)
            x_pack = np.zeros((P, NKS, C), np.float16)
            if n:
                sub = xg[toks, kh * KH:(kh + 1) * KH]      # [n, 2048]
                x_pack[:, :, :n] = sub.reshape(n, NKS, P).transpose(2, 1, 0)
            if n:
                W_kh = w[2 * kh:2 * kh + 2, e].reshape(KH, H)
                w_pack = np.ascontiguousarray(
                    W_kh.reshape(NKS, P, NHC, P).transpose(2, 1, 0, 3)
                ).reshape(NHC, P, H)
            else:
                w_pack = np.zeros((NHC, P, H), np.float16)
            in_maps[c_core][f"x{s}"] = x_pack.reshape(P, NKS * C)
            in_maps[c_core][f"w{s}"] = w_pack

    launches = [in_maps]

    def combine(all_results):
        res = all_results[0]
        y2 = np.zeros((TK, H), np.float32)
        for (c_core, s), (toks, C) in unit_of.items():
            n = len(toks)
            if not n:
                continue
            o_u = res[c_core][f"o{s}"]                     # [4, P, OG*C] f16
            part = o_u.reshape(NHC // OG, P, OG, C)[:, :, :, :n]
            y2[toks] += part.transpose(3, 0, 2, 1).reshape(n, H)
        y = y2.reshape(T, topk, H).sum(axis=1)
        return y.astype(np.float16).reshape(R, T // R, H)

    return nc, launches, combine


def kernel(**inputs) -> np.ndarray:
    nc, launches, combine = prepare(inputs)
    from concourse.bass_utils import run_bass_kernel_spmd

    all_results = []
    for in_maps in launches:
        res = run_bass_kernel_spmd(nc, in_maps, core_ids=list(range(N_CORES)))
        all_results.append(res.results)
    return combine(all_results)


# revision 6
# speedup vs baseline: 1.1364x; 1.1364x over previous
"""MoE top-k routing + grouped down-proj GEMM + reduce-scatter for trn2 (8 cores).

Problem: intermediate_states [4, 2048, 1024] f16 (rank-sharded expanded-token
activations), w [4, 8, 1024, 2048] f16 (rank-sharded per-expert down-proj),
router_logits [1024, 8] f32, topk=2.  Output [4, 256, 2048] f16.

Strategy (W-stationary streaming): per expanded token tk routed to expert
e(tk): y[tk] = gate(tk) * (x_full[tk] @ W_full[e(tk)]) with x_full [TK, 4096]
(rank dim folded into contraction) and W_full[e] [4096, 2048].  Gates are
pre-applied to x on the host (f32 multiply, cast to f16), so the device does
pure GEMM.

Work unit = (expert, K-half, token-chunk<=512).  For 8 experts with c_e<=512
that is 16 units.  Units are sorted by token count and packed into U=2 "slots"
of 8 (one unit per core per slot); each slot has a common padded capacity C_s
= max count in slot, so all 8 cores run an IDENTICAL program (SPMD) perfectly
balanced by construction.  Per unit the PE runs, for each of 16 h-chunks of
128 output columns, a 16-step accumulation over K-subtiles:

    matmul(psum[128h, C], lhsT=W[128k, 128h], rhs=x[128k, C])

i.e. W is the stationary operand (a fresh [128,128] tile per matmul, loaded
via LDWEIGHTS which pipelines ahead of the running matmul through the PE's
reorder window + FWL) and the moving operand is the unit's token block, so
the matmul cost is proportional to the REAL token count (padded only to the
slot capacity), not to a fixed 128-token tile grid.  W is single-use and
streamed through a 6-buffer SBUF pool in exact consumption order (h-major),
~60 GB per-core traffic budget: W 16.8MB + x ~2.4MB + out ~2.2MB.

Evictions (PSUM f32 -> SBUF f16 copy, no scale needed) alternate between the
scalar and vector engines, one per h-chunk, grouped 4 h-chunks per output DMA
(issued on the gpsimd queue to keep the sync queue free for the W stream).
The tail after the last matmul is one eviction + one small DMA.

Host combine: partial outputs are [h, token]-major f16; host transposes,
accumulates the 2 K-half partials and the topk expert partials per token in
f32, and emits [4, 256, 2048] f16.
"""

import numpy as np

R, T_TOK, E = 4, 1024, 8
I_PR, H = 1024, 2048
K = R * I_PR            # 4096 contraction
P = 128
N_CORES = 8
NKS = 16                # K-subtiles per K-half unit (2048/128)
NHC = H // P            # 16 h-chunks
CHUNK_CAP = 512         # max tokens per unit (one PSUM bank of fp32)
OG = 4                  # h-chunks per output DMA group
NPAIR = NHC // 2        # h-chunk pairs (PSUM-bank-alternating matmul order)
NWARM = 6               # p-state warmup matmuls on a zeroed tile
W_BUFS = 4              # W pair-slab SBUF pool depth (4 x 1MB)

_prog_cache: dict[tuple, object] = {}


def _new_bacc():
    from concourse import bacc

    return bacc.Bacc(
        "TRN2",
        target_bir_lowering=False,
        debug=False,
        num_devices=N_CORES,
    )


def _build_program(caps: tuple):
    import concourse.mybir as mybir
    import concourse.tile as tile

    f16 = mybir.dt.float16
    f32 = mybir.dt.float32

    nc = _new_bacc()
    xs, ws, os_ = [], [], []
    for s, C in enumerate(caps):
        xs.append(nc.declare_dram_parameter(f"x{s}", [P, NKS * C], f16,
                                            isOutput=False))
        ws.append(nc.declare_dram_parameter(f"w{s}", [NPAIR, P, 2 * H], f16,
                                            isOutput=False))
        os_.append(nc.declare_dram_parameter(f"o{s}", [NHC // OG, P, OG * C],
                                             f16, isOutput=True))

    with tile.TileContext(nc) as tc:
        with tc.tile_pool(name="sb", bufs=1) as sb, \
             tc.tile_pool(name="ps", bufs=2, space="PSUM") as psp:
            # All DMAs ride the sync HWDGE queue (SWDGE via scalar/gpsimd has
            # multi-us descriptor-generation cost), emitted in consumption
            # order.  Backpressure comes from the W pool rotation (bufs=4),
            # which also paces the out-DMA issue points placed after W issues
            # that are provably later than the source eviction.
            xt = [sb.tile([P, NKS * C], f16, name=f"x{s}", tag=f"x{s}", bufs=1)
                  for s, C in enumerate(caps)]
            wt_of = {}
            ot_of = {}

            def dma_x(s, q):
                C = caps[s]
                XQ = NKS * C // 4
                nc.sync.dma_start(xt[s][:, q * XQ:(q + 1) * XQ],
                                  xs[s][:, q * XQ:(q + 1) * XQ])

            def dma_w(s, pr, quarters=1):
                wt = sb.tile([P, 2 * H], f16, name=f"w{s}_{pr}", tag="w",
                             bufs=W_BUFS)
                wt_of[(s, pr)] = wt
                WQ = 2 * H // quarters
                for q in range(quarters):
                    nc.sync.dma_start(wt[:, q * WQ:(q + 1) * WQ],
                                      ws[s][pr, :, q * WQ:(q + 1) * WQ])

            def dma_o(s, g):
                nc.sync.dma_start(os_[s][g, :, :], ot_of[(s, g)][:])

            U = len(caps)
            # input DMA prologue: first x chunks, first W pair-slab in
            # quarters (so the first matmuls gate on 256KB, not 1MB)
            dma_x(0, 0)
            dma_x(0, 1)
            dma_w(0, 0, quarters=4)
            dma_x(0, 2)
            dma_x(0, 3)
            dma_w(0, 1, quarters=2)
            dma_w(0, 2)
            if U > 1:
                for q in range(4):
                    dma_x(1, q)

            # p-state warmup: PE ramps toward full clock while the first W/x
            # chunks stream in; garbage-free (zeroed operands).
            warm = sb.tile([P, 512], f16, name="warm", tag="warm", bufs=1)
            nc.vector.memset(warm[:], 0.0)
            ps_warm = psp.tile([P, 512], f32, name="psw", tag="ps", bufs=8)
            for i in range(NWARM):
                nc.tensor.matmul(ps_warm[:, :512], lhsT=warm[:, :P],
                                 rhs=warm[:], start=(i == 0),
                                 stop=(i == NWARM - 1))

            # compute: per unit, h-chunk pairs; matmuls alternate between the
            # pair's two PSUM banks so back-to-back matmuls never hit the
            # same bank.  Further W slabs / x tiles / out DMAs are emitted
            # inside the loop (the sync queue issues them in this order,
            # running ahead of compute; the W pool rotation provides
            # backpressure and the WAR deps see the earlier readers).
            pairs_global = [(s, pr) for s in range(U) for pr in range(NPAIR)]
            for t, (s, pr) in enumerate(pairs_global):
                C = caps[s]
                if t + 3 < len(pairs_global):
                    dma_w(*pairs_global[t + 3])
                if pr == 0 and s + 2 < U:
                    for q in range(4):
                        dma_x(s + 2, q)
                if pr == 0 and s > 0:
                    dma_o(s - 1, NHC // OG - 1)
                if pr >= 2 and pr % 2 == 0:
                    dma_o(s, (pr - 2) // 2)
                wt = wt_of[(s, pr)]
                ps_a = psp.tile([P, 512], f32, name=f"psa{s}_{pr}",
                                tag="ps", bufs=8)
                ps_b = psp.tile([P, 512], f32, name=f"psb{s}_{pr}",
                                tag="ps", bufs=8)
                for ks in range(NKS):
                    for half, ps in ((0, ps_a), (1, ps_b)):
                        nc.tensor.matmul(
                            ps[:, :C],
                            lhsT=wt[:, half * H + ks * P:
                                    half * H + (ks + 1) * P],
                            rhs=xt[s][:, ks * C:(ks + 1) * C],
                            start=(ks == 0),
                            stop=(ks == NKS - 1),
                        )
                h0 = 2 * pr
                g, hh0 = divmod(h0, OG)
                if hh0 == 0:
                    ot_of[(s, g)] = sb.tile([P, OG * C], f16,
                                            name=f"o{s}_{g}", tag="o",
                                            bufs=3)
                ot = ot_of[(s, g)]
                nc.scalar.copy(ot[:, hh0 * C:(hh0 + 1) * C], ps_a[:, :C])
                nc.vector.tensor_scalar_mul(
                    ot[:, (hh0 + 1) * C:(hh0 + 2) * C], ps_b[:, :C], 1.0)
            dma_o(U - 1, NHC // OG - 1)
    nc.finalize()
    return nc


def _get_program(caps: tuple):
    if caps not in _prog_cache:
        _prog_cache[caps] = _build_program(caps)
    return _prog_cache[caps]


def _route(logits, topk):
    """numpy replica of jax.lax.top_k + softmax over selected logits."""
    idx = np.argsort(-logits, axis=-1, kind="stable")[:, :topk]      # [T, topk]
    vals = np.take_along_axis(logits, idx, axis=-1)
    mx = vals.max(-1, keepdims=True)
    gate = np.exp(vals - mx)
    gate = gate / gate.sum(-1, keepdims=True)                        # f32
    return idx, gate


def prepare(inputs):
    """Host routing + per-core input construction.

    Returns (nc, launches, combine): launches is a list of per-launch in_maps
    (one dict per core); combine(list_of_per_launch_results) -> final output.
    """
    x = np.asarray(inputs["intermediate_states"])          # [R, TK, I_PR] f16
    w = np.asarray(inputs["w"])                            # [R, E, I_PR, H] f16
    logits = np.asarray(inputs["router_logits"]).astype(np.float32)  # [T, E]
    topk = int(np.asarray(inputs["topk"]))

    T, E_ = logits.shape
    TK = T * topk
    assert x.shape == (R, TK, I_PR) and w.shape == (R, E_, I_PR, H) and E_ == E

    idx, gate = _route(logits, topk)
    flat_e = idx.reshape(-1)                               # expert of tk
    g_flat = gate.reshape(TK)
    counts = np.bincount(flat_e, minlength=E)
    starts = np.zeros(E + 1, np.int64)
    starts[1:] = np.cumsum(counts)
    order = np.argsort(flat_e, kind="stable")              # tks sorted by expert

    # pre-gated activations: y contribution of token row is gate * x row
    xf = np.ascontiguousarray(x.transpose(1, 0, 2)).reshape(TK, K)
    xg = (xf.astype(np.float32) * g_flat[:, None]).astype(np.float16)

    # build units: (expert, kh, token array)
    units = []
    for e in range(E):
        toks_e = order[starts[e]:starts[e + 1]]
        for lo in range(0, max(len(toks_e), 1), CHUNK_CAP):
            chunk = toks_e[lo:lo + CHUNK_CAP]
            for kh in range(2):
                units.append((e, kh, chunk))
    while len(units) % N_CORES:
        units.append((0, 0, np.empty(0, np.int64)))
    units.sort(key=lambda u: -len(u[2]))
    U = len(units) // N_CORES

    caps = []
    for s in range(U):
        cmax = max(len(u[2]) for u in units[s * N_CORES:(s + 1) * N_CORES])
        caps.append(max(8, -(-cmax // 8) * 8))
    caps = tuple(caps)
    nc = _get_program(caps)

    KH = K // 2
    in_maps = [dict() for _ in range(N_CORES)]
    unit_of = {}                                           # (core, slot) -> unit
    for s, C in enumerate(caps):
        for c_core in range(N_CORES):
            e, kh, toks = units[s * N_CORES + c_core]
            n = len(toks)
            unit_of[(c_core, s)] = (toks, C)
            x_pack = np.zeros((P, NKS, C), np.float16)
            if n:
                sub = xg[toks, kh * KH:(kh + 1) * KH]      # [n, 2048]
                x_pack[:, :, :n] = sub.reshape(n, NKS, P).transpose(2, 1, 0)
            if n:
                W_kh = w[2 * kh:2 * kh + 2, e].reshape(KH, H)
                w_pack = np.ascontiguousarray(
                    W_kh.reshape(NKS, P, NPAIR, 2, P)
                    .transpose(2, 1, 3, 0, 4)       # [pair, kpart, half, ks, hcol]
                ).reshape(NPAIR, P, 2 * H)
            else:
                w_pack = np.zeros((NPAIR, P, 2 * H), np.float16)
            in_maps[c_core][f"x{s}"] = x_pack.reshape(P, NKS * C)
            in_maps[c_core][f"w{s}"] = w_pack

    launches = [in_maps]

    def combine(all_results):
        res = all_results[0]
        y2 = np.zeros((TK, H), np.float32)
        for (c_core, s), (toks, C) in unit_of.items():
            n = len(toks)
            if not n:
                continue
            o_u = res[c_core][f"o{s}"]                     # [4, P, OG*C] f16
            part = o_u.reshape(NHC // OG, P, OG, C)[:, :, :, :n]
            y2[toks] += part.transpose(3, 0, 2, 1).reshape(n, H)
        y = y2.reshape(T, topk, H).sum(axis=1)
        return y.astype(np.float16).reshape(R, T // R, H)

    return nc, launches, combine


def kernel(**inputs) -> np.ndarray:
    nc, launches, combine = prepare(inputs)
    from concourse.bass_utils import run_bass_kernel_spmd

    all_results = []
    for in_maps in launches:
        res = run_bass_kernel_spmd(nc, in_maps, core_ids=list(range(N_CORES)))
        all_results.append(res.results)
    return combine(all_results)
